# revision 19
# baseline (speedup 1.0000x reference)
"""MultiHeadAttention + residual + LayerNorm, query-sharded over 8 TRN2 cores.

Sharding: core c -> batch b=c//4, query rows [512*r, 512*(r+1)), r=c%4.
Each core computes K/V projections for its batch's full 2048 tokens
(replicated within the 4-core batch group -> no collectives needed),
attention for its 512 queries over all 16 heads, the output projection,
the residual add, and LayerNorm. Host concatenates the 8 [512,1024] slices.

Token rotation trick: the key/value token order is rotated per-core by
512*r so the core's queries are always columns 0:512 of xT -> identical
program on all cores (SPMD). Softmax is permutation-invariant over keys.

Scores are computed transposed (S^T = K @ Q^T) so no on-chip transposes are
needed anywhere; softmax denominators come free from an appended ones-column
in V (PE accumulates row sums); head pairs share the PE array via
tile_position row tiling (K=64 each).
"""
import numpy as np

B, S, D = 2, 2048, 1024
H, HD = 16, 64
QS = 512          # queries per core
N_CORES = 8
LN_EPS = 1e-12

_cache = {}


def _build(use_gamma, use_beta):
    import concourse.bacc as bacc
    import concourse.tile as tile
    import concourse.bass as bass
    from concourse import mybir

    f32 = mybir.dt.float32
    f16 = mybir.dt.float16
    Exp = mybir.ActivationFunctionType.Exp
    Sqrt = mybir.ActivationFunctionType.Sqrt
    sub = mybir.AluOpType.subtract
    mult = mybir.AluOpType.mult

    nc = bacc.Bacc("TRN2", target_bir_lowering=False, debug=False,
                   num_devices=N_CORES)

    # ---- I/O ----
    xt = nc.dram_tensor("xt", [128, 8, S], f16, kind="ExternalInput")   # x[b]^T k-chunked, tokens rotated
    wq = nc.dram_tensor("wq", [128, 8, D], f16, kind="ExternalInput")   # [D-chunk part, k, d]
    wk = nc.dram_tensor("wk", [128, 8, D], f16, kind="ExternalInput")
    wv = nc.dram_tensor("wv", [128, 8, D], f16, kind="ExternalInput")
    wo = nc.dram_tensor("wo", [128, 8, D], f16, kind="ExternalInput")   # [hd-chunk part, k, n]
    bq = nc.dram_tensor("bq", [128, 8], f32, kind="ExternalInput")
    bk = nc.dram_tensor("bk", [128, 8], f32, kind="ExternalInput")
    bv = nc.dram_tensor("bv", [D], f16, kind="ExternalInput")
    xres = nc.dram_tensor("xres", [128, 4, D], f16, kind="ExternalInput")  # x[b, qrows] + bo, q-chunked
    ident = nc.dram_tensor("ident", [128, 128], f16, kind="ExternalInput")
    if use_gamma:
        gam = nc.dram_tensor("gam", [D], f32, kind="ExternalInput")
    if use_beta:
        bet = nc.dram_tensor("bet", [D], f32, kind="ExternalInput")
    out = nc.dram_tensor("out", [QS, D], f32, kind="ExternalOutput")

    NT = S // 128     # 16 token chunks
    NK = 8            # contraction chunks of 128 over D
    NM = 8            # hd m-chunks of 128 (2 heads each)
    KC = S // 128     # 16 key chunks

    with tile.TileContext(nc) as tc:
        with (
            tc.tile_pool(name="big", bufs=1) as big,
            tc.tile_pool(name="wpool", bufs=3) as wpool,
            tc.tile_pool(name="pt", bufs=3) as ptp,
            tc.tile_pool(name="small", bufs=2) as small,
            tc.tile_pool(name="norm", bufs=2) as normp,
            tc.tile_pool(name="dram", bufs=4, space="DRAM") as dram,
            # PSUM: pa 2x1 + sps 2x2 + cxs 2x1 = 8 banks, statically coexist
            tc.tile_pool(name="pa", bufs=2, space="PSUM") as pa,
            tc.tile_pool(name="sps", bufs=2, space="PSUM") as sps,
            tc.tile_pool(name="cxs", bufs=2, space="PSUM") as cxs,
        ):
            # ---------- loads (priority-ordered, chunked for early start) ----
            xt_sb = big.tile([128, 8, S], f16)
            wq_sb = wpool.tile([128, 8, D], f16, tag="w")
            wk_sb = wpool.tile([128, 8, D], f16, tag="w")
            wv_sb = wpool.tile([128, 8, D], f16, tag="w")
            bq_sb = big.tile([128, 8], f32)
            bk_sb = big.tile([128, 8], f32)
            bv_bc = big.tile([128, D], f16)
            eps_sb = big.tile([128, 1], f32)
            nc.vector.memset(eps_sb[:], LN_EPS)
            # wq full first (all 8 QT chains progress per xt chunk), then wk, wv
            nc.sync.dma_start(out=wq_sb[:, 0, :], in_=wq[:, 0, :])
            nc.sync.dma_start(out=xt_sb[:, 0, 0:QS], in_=xt[:, 0, 0:QS])
            nc.sync.dma_start(out=xt_sb[:, 0, QS:S], in_=xt[:, 0, QS:S])
            for k in range(1, NK):
                nc.sync.dma_start(out=wq_sb[:, k, :], in_=wq[:, k, :])
                nc.sync.dma_start(out=xt_sb[:, k, :], in_=xt[:, k, :])
            nc.sync.dma_start(out=bq_sb[:], in_=bq[:])
            nc.sync.dma_start(out=bk_sb[:], in_=bk[:])
            nc.sync.dma_start(
                out=bv_bc[:],
                in_=bass.AP(tensor=bv, offset=0, ap=[[0, 128], [1, D]]))
            for k in range(NK):
                nc.sync.dma_start(out=wk_sb[:, k, :], in_=wk[:, k, :])
            for k in range(NK):
                nc.sync.dma_start(out=wv_sb[:, k, :], in_=wv[:, k, :])

            id_sb = big.tile([128, 128], f16)
            nc.sync.dma_start(out=id_sb[:], in_=ident[:])
            QT = [big.tile([128, QS], f16, name=f"QT{i}") for i in range(NM)]
            KT = [big.tile([128, S], f16, name=f"KT{i}") for i in range(NM)]
            V = [big.tile([128, NT, 8, 65], f16, name=f"V{i}") for i in range(2)]
            for n in range(2):
                nc.vector.memset(V[n][:, :, :, 64:65], 1.0)
            ctxT = [big.tile([128, QS], f16, name=f"ctxT{i}") for i in range(NM)]
            ones64 = big.tile([128, 64], f32)
            nc.vector.memset(ones64[64:65, :], 1.0)

            # ---------- emission helpers ----------
            def proj_qt(m, pool=None):
                ps = (pool or pa).tile([128, QS], f32, tag=(pool or pa).name)
                for k in range(NK):
                    nc.tensor.matmul(
                        ps[:], wq_sb[:, k, m * 128:(m + 1) * 128],
                        xt_sb[:, k, 0:QS],
                        start=(k == 0), stop=(k == NK - 1))
                nc.vector.tensor_scalar_add(
                    out=QT[m][:], in0=ps[:], scalar1=bq_sb[:, m:m + 1])

            def proj_kt(m, t, pool=None):
                ps = (pool or pa).tile([128, QS], f32, tag=(pool or pa).name)
                for k in range(NK):
                    nc.tensor.matmul(
                        ps[:], wk_sb[:, k, m * 128:(m + 1) * 128],
                        xt_sb[:, k, t * 512:(t + 1) * 512],
                        start=(k == 0), stop=(k == NK - 1))
                nc.vector.tensor_scalar_add(
                    out=KT[m][:, t * 512:(t + 1) * 512], in0=ps[:],
                    scalar1=bk_sb[:, m:m + 1])

            def proj_v(n, t, pool=None):
                ps = (pool or pa).tile([128, QS], f32, tag=(pool or pa).name)
                for k in range(NK):
                    nc.tensor.matmul(
                        ps[:], xt_sb[:, k, t * 128:(t + 1) * 128],
                        wv_sb[:, k, n * 512:(n + 1) * 512],
                        start=(k == 0), stop=(k == NK - 1))
                nc.vector.tensor_add(
                    out=V[n][:, t, :, 0:64],
                    in0=ps.rearrange("p (h d) -> p h d", h=8),
                    in1=bv_bc[:, n * 512:(n + 1) * 512].rearrange(
                        "p (h d) -> p h d", h=8))

            def attn(hp, fillers=()):
                fillers = list(fillers)
                n = hp // 4
                h0 = (2 * hp) % 8
                cx0 = cxs.tile([65, QS], f32, tag="cxs", name="cx0")
                cx1 = cxs.tile([65, QS], f32, tag="cxs", name="cx1")
                for kc in range(KC):
                    ks = slice(kc * 128, (kc + 1) * 128)
                    sp = sps.tile([128, 2 * QS], f32, tag="sps", name="sp")
                    nc.tensor.matmul(
                        sp[:, 0:QS], KT[hp][0:64, ks], QT[hp][0:64, :],
                        start=True, stop=True, tile_position=(0, 0))
                    nc.tensor.matmul(
                        sp[:, QS:2 * QS], KT[hp][64:128, ks],
                        QT[hp][64:128, :],
                        start=True, stop=True, tile_position=(64, 0))
                    pt = ptp.tile([128, 2 * QS], f16, tag="pt")
                    nc.scalar.activation(out=pt[:], in_=sp[:], func=Exp,
                                         scale=0.125)
                    nc.tensor.matmul(
                        cx0[:], V[n][:, kc, h0, :], pt[:, 0:QS],
                        start=(kc == 0), stop=(kc == KC - 1))
                    nc.tensor.matmul(
                        cx1[:], V[n][:, kc, h0 + 1, :], pt[:, QS:2 * QS],
                        start=(kc == 0), stop=(kc == KC - 1))
                    for _ in range(4):
                        if fillers:
                            fillers.pop(0)()
                for f in fillers:
                    f()
                for i, cx in ((0, cx0), (1, cx1)):
                    rsb = small.tile([65, QS], f32, tag="rsb")
                    nc.vector.reciprocal(out=rsb[64:65, :], in_=cx[64:65, :])
                    rb = small.tile([64, QS], f32, tag="rb")
                    if hp == NM - 1:
                        # tail-latency path: PE outer-product broadcast
                        bc = pa.tile([64, QS], f32, tag="pa", name="bc")
                        nc.tensor.matmul(bc[:], ones64[64:65, :],
                                         rsb[64:65, :], start=True, stop=True,
                                         tile_position=(64, 0))
                        nc.scalar.copy(out=rb[:], in_=bc[:])
                    else:
                        sc = dram.tile([QS], f32, tag="scr")
                        nc.sync.dma_start(out=sc[:], in_=rsb[64:65, :])
                        nc.sync.dma_start(
                            out=rb[:],
                            in_=bass.AP(tensor=sc.tensor, offset=sc.offset,
                                        ap=[[0, 64], [1, QS]]))
                    nc.vector.tensor_mul(out=ctxT[hp][i * 64:(i + 1) * 64, :],
                                         in0=cx[0:64, :], in1=rb[:])

            # ---------- emission: interleave proj with attention ----------
            pools3 = [pa, cxs, sps]
            for m in range(NM):
                proj_qt(m, pools3[m % 3])
            for t in range(4):
                proj_kt(0, t, pools3[t % 3])
            for t in range(NT):
                proj_v(0, t, pools3[t % 3])
            attn(0)
            wo_sb = wpool.tile([128, 8, D], f16, tag="w")
            for k in range(NK):
                nc.sync.dma_start(out=wo_sb[:, k, :], in_=wo[:, k, :])
            for m in range(1, 4):
                for t in range(4):
                    proj_kt(m, t)
                attn(m)
            for t in range(NT):
                proj_v(1, t)
            for t in range(4):
                proj_kt(4, t)
            attn(4)
            for m in range(5, NM - 1):
                for t in range(4):
                    proj_kt(m, t)
                attn(m)
            for t in range(4):
                proj_kt(NM - 1, t)

            attn(NM - 1)

            if use_gamma:
                gb = big.tile([128, D], f32)
                nc.sync.dma_start(
                    out=gb[:],
                    in_=bass.AP(tensor=gam, offset=0, ap=[[0, 128], [1, D]]))
            if use_beta:
                bb = big.tile([128, D], f32)
                nc.sync.dma_start(
                    out=bb[:],
                    in_=bass.AP(tensor=bet, offset=0, ap=[[0, 128], [1, D]]))

            # ---------- output projection (+residual via identity mm) + LN ----
            for q in range(4):
                qs = slice(q * 128, (q + 1) * 128)
                xr16 = normp.tile([128, D], f16, tag="xr16")
                nc.sync.dma_start(out=xr16[:], in_=xres[:, q, :])
                h_sb = normp.tile([128, D], f32, tag="h")
                for nn in range(2):
                    ns = slice(nn * 512, (nn + 1) * 512)
                    ypool = pools3[(2 * q + nn) % 3]
                    yp = ypool.tile([128, 512], f32, tag=ypool.name, name="yp")
                    for k in range(NM):
                        nc.tensor.matmul(
                            yp[:], ctxT[k][:, qs], wo_sb[:, k, ns],
                            start=(k == 0), stop=False)
                    nc.tensor.matmul(yp[:], id_sb[:], xr16[:, ns],
                                     start=False, stop=True)
                    nc.scalar.copy(out=h_sb[:, ns], in_=yp[:])
                stats = normp.tile([128, 2, 6], f32, tag="st")
                for i in range(2):
                    nc.vector.bn_stats(
                        out=stats[:, i, :],
                        in_=h_sb[:, i * 512:(i + 1) * 512])
                mv = normp.tile([128, 2], f32, tag="mv")
                nc.vector.bn_aggr(out=mv[:], in_=stats[:])
                rstd = normp.tile([128, 4], f32, tag="rs")
                nc.scalar.activation(out=rstd[:, 0:1], in_=mv[:, 1:2],
                                     func=Sqrt, bias=eps_sb[:, 0:1])
                nc.vector.reciprocal(out=rstd[:, 1:2], in_=rstd[:, 0:1])
                # nmb = -mean * rstd; out = h * rstd + nmb   (on ScalarE)
                nc.vector.tensor_tensor(
                    out=rstd[:, 2:3], in0=mv[:, 0:1], in1=rstd[:, 1:2],
                    op=mult)
                nc.scalar.mul(out=rstd[:, 3:4], in_=rstd[:, 2:3], mul=-1.0)
                o_sb = h_sb
                nc.scalar.activation(
                    out=o_sb[:], in_=h_sb[:],
                    func=mybir.ActivationFunctionType.Identity,
                    bias=rstd[:, 3:4], scale=rstd[:, 1:2])
                if use_gamma:
                    nc.vector.tensor_mul(out=o_sb[:], in0=o_sb[:], in1=gb[:])
                if use_beta:
                    nc.vector.tensor_add(out=o_sb[:], in0=o_sb[:], in1=bb[:])
                nc.sync.dma_start(out=out[qs, :], in_=o_sb[:])

    nc.compile()
    return nc


def _chunk_k(w):
    # [D, D] -> [128, 8, D]: partition = D%128 within chunk, free = (chunk, col)
    return np.ascontiguousarray(
        w.reshape(8, 128, w.shape[1]).transpose(1, 0, 2))


def kernel(x, wq, bq, wk, bk, wv, bv, wo, bo, gamma, beta):
    x = np.asarray(x, np.float32)
    wq, bq = np.asarray(wq, np.float32), np.asarray(bq, np.float32)
    wk, bk = np.asarray(wk, np.float32), np.asarray(bk, np.float32)
    wv, bv = np.asarray(wv, np.float32), np.asarray(bv, np.float32)
    wo, bo = np.asarray(wo, np.float32), np.asarray(bo, np.float32)
    gamma, beta = np.asarray(gamma, np.float32), np.asarray(beta, np.float32)

    use_gamma = not np.allclose(gamma, 1.0)
    use_beta = not np.allclose(beta, 0.0)

    key = (use_gamma, use_beta)
    if key not in _cache:
        _cache[key] = _build(use_gamma, use_beta)
    nc = _cache[key]

    wq16, wk16, wv16 = (_chunk_k(w.astype(np.float16)) for w in (wq, wk, wv))
    wo16 = _chunk_k(wo.astype(np.float16))
    bq8 = np.ascontiguousarray(bq.reshape(8, 128).T)
    bk8 = np.ascontiguousarray(bk.reshape(8, 128).T)

    in_maps = []
    for c in range(N_CORES):
        b, r = c // 4, c % 4
        xt = x[b].T.astype(np.float16)                      # [D, S]
        xt_rot = np.concatenate([xt[:, 512 * r:], xt[:, :512 * r]], axis=1)
        xt_c = np.ascontiguousarray(
            xt_rot.reshape(8, 128, S).transpose(1, 0, 2))
        m = {
            "xt": xt_c, "wq": wq16, "wk": wk16, "wv": wv16, "wo": wo16,
            "bq": bq8, "bk": bk8, "bv": bv.astype(np.float16),
            "xres": np.ascontiguousarray(
                (x[b, 512 * r:512 * (r + 1), :] + bo).astype(np.float16)
                .reshape(4, 128, D).transpose(1, 0, 2)),
            "ident": np.eye(128, dtype=np.float16),
        }
        if use_gamma:
            m["gam"] = gamma
        if use_beta:
            m["bet"] = beta
        in_maps.append(m)

    from concourse.bass_utils import run_bass_kernel_spmd
    res = run_bass_kernel_spmd(nc, in_maps, list(range(N_CORES)))

    out = np.empty((B, S, D), np.float32)
    for c in range(N_CORES):
        b, r = c // 4, c % 4
        out[b, 512 * r:512 * (r + 1), :] = res.results[c]["out"]
    return out
